# revision 56
# baseline (speedup 1.0000x reference)
"""Multi-head causal attention (B=4, C=2048, E=1024, H=16, D=64) on 8 trn2 cores.

Sharding: core i = (batch b=i//2, head-group g=i%2).  Each core computes its
batch's attention for 8 heads (512 features) and a partial output projection;
the host sums the two partials per batch (W_o split row-wise).

Single software-pipelined pass, qc-outer / head-pair-inner.  The PE stream for
each attention group (qc, hp) is  [S(kb+1) | filler | AV(kb)]  so the scalar
engine's exp latency is hidden; projection matmuls (V, Q, K for upcoming
groups) and the output projection of the previous q-chunk are pumped into the
stream as filler, so the tensor engine never idles waiting on exp/normalize.
Hidden states stay in SBUF between attention and the output projection (no
DRAM roundtrip).  Diagonal k-blocks only compute the causally-valid q-window
(the S matmul window is floored at 256 columns: fp32r drops to 1/4 rate below
an output free size of 256).
"""

import contextlib
from collections import deque

import numpy as np

import concourse.bass as bass
import concourse.mybir as mybir
import concourse.tile as tile
from concourse.vector_clock import ScopedClock

B, C, E = 4, 2048, 1024
H, D = 16, 64
N_CORES = 8
GF = 512          # features per head-group (8 heads x 64)
HP = 4            # head-pairs per group
QC = 512          # q-chunk width
KB = 128          # k-block width
NQC = C // QC     # 4
NKB = C // KB     # 16
NE = E // 128     # 8 contraction tiles over E
F32 = mybir.dt.float32
F32R = mybir.dt.float32r
BF16 = mybir.dt.bfloat16

# 0: all matmuls full-width (baseline shapes, scheduling change only)
# 1: trim exp/mask/AV to the causally-valid q-window; S full width
# 2: additionally trim the fp32r S matmuls on dr>=2 diagonal blocks to a
#    256-col window (N=384/offset-128 shapes from the full trim crashed
#    the exec unit; 256@256 is the conservative subset)
TRIM = 2

# Q/K projections as fp8e4m3 DoubleRow matmuls (K=256 per call, 2x rate).
# Host pre-packs x and 32*Wq/Wk with E-pairs along the contraction dim;
# the 32x weight scale is compensated in the exp's scale argument.
USE_FP8_QK = True
FP8 = mybir.dt.float8e4

# AluOpType.divide fails the walrus backend verifier, so normalization
# stays on ACT via exp(-ln(rs)).  To keep it off the group-boundary
# critical path, hid is first copied raw to SBUF (one DVE copy frees the
# PSUM tile) and the ln/exp/multiply chain is deferred into the next
# group's kb loop.
USE_DVE_DIV = False

_CACHED_NC = None


class PatchedTC(tile.TileContext):
    """This walrus build caps sync waits per instruction (1 for CTRL, ~2 for
    compute ISA structs).  Hoist excess waits onto same-engine NOPs emitted
    just before the instruction (engine streams execute in order, so the
    semantics are identical), and split the end-of-kernel drain's waits
    across single-wait drain instructions."""

    WAIT_CAP = 1

    def _commit_instruction(self, inst, lazy_reg_writes=True):
        si = getattr(inst, "sync_info", None)
        if (
            si is not None
            and len(si.on_wait) > self.WAIT_CAP
            and getattr(inst, "engine", mybir.EngineType.Unassigned)
            != mybir.EngineType.Unassigned
        ):
            waits = list(si.on_wait)
            keep = waits[: self.WAIT_CAP]
            extra = waits[self.WAIT_CAP :]
            si.on_wait[:] = keep
            for w in extra:
                nop = mybir.InstNoOp(
                    name=f"I-nw{self.nc.next_id()}",
                    engine=inst.engine,
                    bass_nofuse=True,
                    sync_info=mybir.SyncInfo(on_wait=[w], on_update=[]),
                )
                super()._commit_instruction(nop, lazy_reg_writes=False)
        return super()._commit_instruction(inst, lazy_reg_writes)

    def _drain_and_barrier(self, tick_clock, wait_clock):
        carrier = self.nc.sync.drain()
        wait_clock.add_sem_waits(
            carrier.ins, ScopedClock({None: tick_clock.global_clock})
        )
        si = carrier.ins.sync_info
        waits = list(si.on_wait) if si is not None else []
        if len(waits) > 1:
            si.on_wait[:] = waits[:1]
            for w in waits[1:]:
                extra = self.nc.sync.drain()
                extra.ins.sync_info = mybir.SyncInfo(on_wait=[w], on_update=[])
        self.nc.all_engine_barrier()
        assert self.sems is not None
        popped = self.nc._tile_sem_poison_stack.pop()
        assert popped is self._sem_poison
        self.nc.clear_and_free_semaphores(list(self.sems.allocated().values()))
        self.nc.all_engine_barrier()


def build_nc():
    nc = bass.Bass("TRN2", target_bir_lowering=False)
    xT = nc.declare_dram_parameter("xT", [E, C], BF16, isOutput=False)
    Wq = nc.declare_dram_parameter("Wq", [E, GF], BF16, isOutput=False)
    Wk = nc.declare_dram_parameter("Wk", [E, GF], BF16, isOutput=False)
    Wv = nc.declare_dram_parameter("Wv", [E, GF], BF16, isOutput=False)
    Wo = nc.declare_dram_parameter("Wo", [GF, E], BF16, isOutput=False)
    tri = nc.declare_dram_parameter("tri", [128, 256], BF16, isOutput=False)
    msk = nc.declare_dram_parameter("mask", [128, 4 * QC], BF16, isOutput=False)
    if USE_FP8_QK:
        xq8 = nc.declare_dram_parameter("xq8", [128, 4, 2, C], FP8, isOutput=False)
        wq8 = nc.declare_dram_parameter("wq8", [128, 4, 2, GF], FP8, isOutput=False)
        wk8 = nc.declare_dram_parameter("wk8", [128, 4, 2, GF], FP8, isOutput=False)
    out = nc.declare_dram_parameter("out", [C, E], F32, isOutput=True)

    xT_t = xT.ap().rearrange("(po pi) f -> pi po f", pi=128)    # [128, 8, C]
    Wq_t = Wq.ap().rearrange("(po pi) f -> pi po f", pi=128)    # [128, 8, GF]
    Wk_t = Wk.ap().rearrange("(po pi) f -> pi po f", pi=128)
    Wv_t = Wv.ap().rearrange("(po pi) f -> pi po f", pi=128)
    Wo_t = Wo.ap().rearrange("(po pi) f -> pi po f", pi=128)    # [128, 4, E]

    with PatchedTC(nc) as tc:
        with contextlib.ExitStack() as ctx:
            consts = ctx.enter_context(tc.tile_pool(name="consts", bufs=1))
            xpool = ctx.enter_context(tc.tile_pool(name="xpool", bufs=1))
            wpool = ctx.enter_context(tc.tile_pool(name="wpool", bufs=1))
            vpool = ctx.enter_context(tc.tile_pool(name="vpool", bufs=1))
            ktpool = ctx.enter_context(tc.tile_pool(name="ktpool", bufs=1))
            qtpool = ctx.enter_context(tc.tile_pool(name="qtpool", bufs=2))
            wtpool = ctx.enter_context(tc.tile_pool(name="wtpool", bufs=2))
            napool = ctx.enter_context(tc.tile_pool(name="napool", bufs=2))
            rawpool = ctx.enter_context(tc.tile_pool(name="rawpool", bufs=2))
            stgpool = ctx.enter_context(tc.tile_pool(name="stgpool", bufs=1))
            ospool = ctx.enter_context(tc.tile_pool(name="ospool", bufs=3))
            stpool = ctx.enter_context(
                tc.tile_pool(name="stpsum", bufs=2, space="PSUM")
            )
            hidpool = ctx.enter_context(
                tc.tile_pool(name="hidpsum", bufs=1, space="PSUM")
            )
            filpool = ctx.enter_context(
                tc.tile_pool(name="filpsum", bufs=2, space="PSUM")
            )

            # DMA issue order = first-needed-first: the prologue is gated on
            # wv + the first xT token-chunk (V projection), then wq/wk for
            # the qc=0 kt/qt; wo isn't consumed until the first output
            # projection (~80us in).  DMA queue rings start staggered by
            # ~200ns each, so the first ~16 issues land on the earliest
            # queues — they must be exactly the chunks the first V chain
            # consumes, interleaved in consumption order.
            tri_sb = consts.tile([128, 256], BF16)
            mask_sb = consts.tile([128, 4 * QC], BF16)
            if TRIM == 0:
                nc.sync.dma_start(mask_sb[:], msk.ap())

            wv_sb = wpool.tile([128, NE, GF], BF16, tag="wv")
            wq_sb = wk_sb = None
            if not USE_FP8_QK:
                wq_sb = wpool.tile([128, NE, GF], BF16, tag="wq")
                wk_sb = wpool.tile([128, NE, GF], BF16, tag="wk")
            wo_sb = wpool.tile([128, HP, E], BF16, tag="wo")
            xT_sb = xpool.tile([128, NE, C], BF16)

            if USE_FP8_QK:
                xq8_sb = xpool.tile([128, 4, 2, C], FP8)
                wq8_sb = wpool.tile([128, 4, 2, GF], FP8, tag="wq8")
                wk8_sb = wpool.tile([128, 4, 2, GF], FP8, tag="wk8")

            # chunk every load to ~128-256KB so it spreads across the 16 DMA
            # queues; a monolithic 1MB DMA serializes on a single queue.
            for e in range(NE):
                nc.sync.dma_start(wv_sb[:, e, :], Wv_t[:, e, :])
                nc.sync.dma_start(xT_sb[:, e, 0:QC], xT_t[:, e, 0:QC])
            nc.sync.dma_start(tri_sb[:], tri.ap())
            if USE_FP8_QK:
                for e4 in range(4):
                    nc.sync.dma_start(wk8_sb[:, e4, :, :], wk8.ap()[:, e4, :, :])
                for e4 in range(4):
                    nc.sync.dma_start(wq8_sb[:, e4, :, :], wq8.ap()[:, e4, :, :])
            else:
                for e in range(NE):
                    nc.sync.dma_start(wk_sb[:, e, :], Wk_t[:, e, :])
                    nc.sync.dma_start(wq_sb[:, e, :], Wq_t[:, e, :])
            for t in range(NQC):
                if t > 0:
                    for e in range(NE):
                        nc.sync.dma_start(
                            xT_sb[:, e, t * QC : (t + 1) * QC],
                            xT_t[:, e, t * QC : (t + 1) * QC],
                        )
                if USE_FP8_QK:
                    nc.sync.dma_start(
                        xq8_sb[:, :, :, t * QC : (t + 1) * QC],
                        xq8.ap()[:, :, :, t * QC : (t + 1) * QC],
                    )
            for f in range(HP):
                nc.sync.dma_start(wo_sb[:, f, :], Wo_t[:, f, :])

            # v_sb: per head 128 cols = [64 V feats | 64 ones] so the AV
            # matmul's M rows 64:128 accumulate the softmax denominator for
            # free.  Only the ones-halves need the memset; V halves are
            # overwritten by the projection copies.
            v_sb = vpool.tile([128, NKB, 2 * GF], BF16)
            ones_ap = v_sb[:].rearrange("p t (h u) -> p t h u", u=128)[
                :, :, :, 64:128
            ]
            nc.any.memset(ones_ap, 1.0)

            kt_sb = ktpool.tile([128, HP, C], F32R)
            stage_sb = stgpool.tile([128, 2, HP, QC], BF16)

            qts = {}
            pending_norm = []

            def flush_norm():
                # deferred softmax normalization: reads the raw hid copy, so
                # the issue point is decoupled from the PSUM lifetime
                while pending_norm:
                    raw, nqc, nhp = pending_norm.pop(0)
                    lnr = napool.tile([64, 2 * QC], F32, tag="ln")
                    rec = napool.tile([64, 2 * QC], F32, tag="rec")
                    nc.scalar.activation(
                        lnr[:], raw[64:128, :], mybir.ActivationFunctionType.Ln
                    )
                    nc.scalar.activation(
                        rec[:], lnr[:], mybir.ActivationFunctionType.Exp,
                        scale=-1.0,
                    )
                    nc.vector.tensor_tensor(
                        stage_sb[0:64, nqc % 2, nhp, :],
                        raw[0:64, 0:QC],
                        rec[:, 0:QC],
                        mybir.AluOpType.mult,
                    )
                    nc.vector.tensor_tensor(
                        stage_sb[64:128, nqc % 2, nhp, :],
                        raw[0:64, QC : 2 * QC],
                        rec[:, QC : 2 * QC],
                        mybir.AluOpType.mult,
                    )

            # ---- filler tasks: generators yielding once per PE matmul.
            def task_v(tb):
                fil = filpool.tile([128, QC], F32, tag="fil")
                for e in range(NE):
                    nc.tensor.matmul(
                        fil[:],
                        lhsT=xT_sb[:, e, tb * 128 : (tb + 1) * 128],
                        rhs=wv_sb[:, e, :],
                        start=(e == 0),
                        stop=(e == NE - 1),
                    )
                    yield
                dst = v_sb[:, tb, :].rearrange("p (h u) -> p h u", u=128)[
                    :, :, 0:64
                ]
                nc.vector.tensor_copy(
                    dst, fil[:].rearrange("p (h u) -> p h u", u=64)
                )

            def _proj_chain(fil, w8_sb, w_sb, hp, tok0):
                if USE_FP8_QK:
                    for e4 in range(4):
                        nc.tensor.matmul(
                            fil[:],
                            lhsT=w8_sb[:, e4, :, hp * 128 : (hp + 1) * 128],
                            rhs=xq8_sb[:, e4, :, tok0 : tok0 + QC],
                            start=(e4 == 0),
                            stop=(e4 == 3),
                            perf_mode=mybir.MatmulPerfMode.DoubleRow,
                        )
                        yield
                else:
                    for e in range(NE):
                        nc.tensor.matmul(
                            fil[:],
                            lhsT=w_sb[:, e, hp * 128 : (hp + 1) * 128],
                            rhs=xT_sb[:, e, tok0 : tok0 + QC],
                            start=(e == 0),
                            stop=(e == NE - 1),
                        )
                        yield

            def task_qt(qc, hp):
                fil = filpool.tile([128, QC], F32, tag="fil")
                yield from _proj_chain(
                    fil, wq8_sb if USE_FP8_QK else None, wq_sb, hp, qc * QC
                )
                qt = qtpool.tile([128, QC], F32R, tag="qt")
                nc.vector.tensor_copy(qt[:], fil[:])
                qts[(qc, hp)] = qt

            def task_kt(qc1, hp):
                fil = filpool.tile([128, QC], F32, tag="fil")
                yield from _proj_chain(
                    fil, wk8_sb if USE_FP8_QK else None, wk_sb, hp, qc1 * QC
                )
                nc.vector.tensor_copy(
                    kt_sb[:, hp, qc1 * QC : (qc1 + 1) * QC], fil[:]
                )

            def task_o(qcm, j):
                # output projection for row-block 4*qcm+j of q-chunk qcm;
                # the two E-half chains interleave so a late stage (the last
                # group's deferred norm) stalls as few matmuls as possible
                qb = 4 * qcm + j
                fils = [
                    filpool.tile([128, QC], F32, tag="fil", name=f"ofil{ec}")
                    for ec in range(2)
                ]
                for f in range(HP):
                    for ec in range(2):
                        nc.tensor.matmul(
                            fils[ec][:],
                            lhsT=stage_sb[:, qcm % 2, f, j * 128 : (j + 1) * 128],
                            rhs=wo_sb[:, f, ec * QC : (ec + 1) * QC],
                            start=(f == 0),
                            stop=(f == HP - 1),
                        )
                        yield
                for ec in range(2):
                    so = ospool.tile([128, QC], F32, tag="so")
                    nc.vector.tensor_copy(so[:], fils[ec][:])
                    nc.sync.dma_start(
                        out.ap()[qb * 128 : (qb + 1) * 128, ec * QC : (ec + 1) * QC],
                        so[:],
                    )

            tasks = deque()

            def pump(n):
                done = 0
                while done < n and tasks:
                    try:
                        next(tasks[0][1])
                        done += 1
                    except StopIteration:
                        tasks.popleft()

            def drain_older(gidx):
                while tasks and tasks[0][0] < gidx:
                    try:
                        next(tasks[0][1])
                    except StopIteration:
                        tasks.popleft()

            # prologue work: V for k-blocks of qc=0, kt for qc=0, qt(0,0)
            for tb in range(4):
                tasks.append((-1, task_v(tb)))
            for hp in range(HP):
                tasks.append((-1, task_kt(0, hp)))
            tasks.append((-1, task_qt(0, 0)))

            # ---- main loop: attention groups with interleaved filler
            gidx = 0
            for qc in range(NQC):
                nkb = 4 * qc + 4
                for hp in range(HP):
                    # queue filler consumed by upcoming groups
                    if hp < HP - 1:
                        tasks.append((gidx, task_qt(qc, hp + 1)))
                    elif qc < NQC - 1:
                        tasks.append((gidx, task_qt(qc + 1, 0)))
                    if qc < NQC - 1:
                        tasks.append((gidx, task_kt(qc + 1, hp)))
                        tasks.append((gidx, task_v(4 * (qc + 1) + hp)))
                    if qc > 0:
                        tasks.append((gidx, task_o(qc - 1, hp)))

                    drain_older(gidx)
                    qt = qts.pop((qc, hp))
                    hid = hidpool.tile([128, 2 * QC], F32, tag="hid")
                    pend = {}
                    for kb in range(nkb + 1):
                        if kb < nkb:
                            dr = kb - 4 * qc
                            # causally-valid q-window of this k-block (the
                            # S window is floored at 256 cols: fp32r is 1/4
                            # rate below an output free size of 256)
                            qoff_s = 256 if TRIM >= 2 and dr >= 2 else 0
                            qoff_a = dr * 128 if TRIM >= 1 and dr >= 1 else 0
                            st = stpool.tile([128, 2 * QC], F32, tag="st")
                            nc.tensor.matmul(
                                st[:, qoff_s:QC],
                                lhsT=kt_sb[0:64, hp, kb * KB : (kb + 1) * KB],
                                rhs=qt[0:64, qoff_s:QC],
                                start=True,
                                stop=True,
                            )
                            nc.tensor.matmul(
                                st[:, QC + qoff_s : 2 * QC],
                                lhsT=kt_sb[64:128, hp, kb * KB : (kb + 1) * KB],
                                rhs=qt[64:128, qoff_s:QC],
                                start=True,
                                stop=True,
                            )
                            wt = wtpool.tile([128, 2 * QC], BF16, tag="wt")
                            if TRIM >= 1:
                                st3 = st[:].rearrange("p (a q) -> p a q", a=2)[
                                    :, :, qoff_a:QC
                                ]
                                wt3 = wt[:].rearrange("p (a q) -> p a q", a=2)[
                                    :, :, qoff_a:QC
                                ]
                            else:
                                st3, wt3 = st[:], wt[:]
                            nc.scalar.activation(
                                wt3, st3, mybir.ActivationFunctionType.Exp,
                                scale=0.125 / 1024.0 if USE_FP8_QK else 0.125,
                            )
                            if dr >= 0:
                                if TRIM >= 1:
                                    # intra-block causal staircase
                                    mw = 128
                                    msrc = tri_sb[:, None, 128:256]
                                else:
                                    mw = QC
                                    msrc = mask_sb[:, None, dr * QC : (dr + 1) * QC]
                                wtm = wt[:].rearrange("p (a q) -> p a q", a=2)[
                                    :, :, qoff_a : qoff_a + mw
                                ]
                                nc.vector.tensor_tensor(
                                    wtm,
                                    wtm,
                                    msrc.to_broadcast((128, 2, mw)),
                                    mybir.AluOpType.mult,
                                )
                            pend[kb] = (wt, qoff_a)
                        if kb == 1:
                            # previous group's normalization: issued after
                            # this group's first exps so it doesn't delay
                            # them in the ACT queue
                            flush_norm()
                        # pump rate tapers with qc so late groups keep a
                        # filler cushion for the group-boundary WAR gap
                        # (qc=3 holds everything back for the boundaries)
                        if kb >= nkb:
                            pump(4)
                        elif qc == 3:
                            pump(0)
                        else:
                            pump((2, 2, 1)[qc])
                        if kb >= 1:
                            wtp, qoffp = pend.pop(kb - 1)
                            kbp = kb - 1
                            nc.tensor.matmul(
                                hid[:, qoffp:QC],
                                lhsT=v_sb[:, kbp, 2 * hp * 128 : (2 * hp + 1) * 128],
                                rhs=wtp[:, qoffp:QC],
                                start=(kbp == 0),
                                stop=(kbp == nkb - 1),
                                skip_group_check=True,
                            )
                            nc.tensor.matmul(
                                hid[:, QC + qoffp : 2 * QC],
                                lhsT=v_sb[
                                    :, kbp, (2 * hp + 1) * 128 : (2 * hp + 2) * 128
                                ],
                                rhs=wtp[:, QC + qoffp : 2 * QC],
                                start=(kbp == 0),
                                stop=(kbp == nkb - 1),
                                skip_group_check=True,
                            )

                    if qc == NQC - 1 and hp == HP - 1:
                        # last group: normalize directly from hid (nothing
                        # follows that needs the PSUM tile, and the epilogue
                        # output projection wants the stage ASAP)
                        lnr = napool.tile([64, 2 * QC], F32, tag="ln")
                        rec = napool.tile([64, 2 * QC], F32, tag="rec")
                        nc.scalar.activation(
                            lnr[:], hid[64:128, :],
                            mybir.ActivationFunctionType.Ln,
                        )
                        nc.scalar.activation(
                            rec[:], lnr[:], mybir.ActivationFunctionType.Exp,
                            scale=-1.0,
                        )
                        nc.vector.tensor_tensor(
                            stage_sb[0:64, qc % 2, hp, :],
                            hid[0:64, 0:QC],
                            rec[:, 0:QC],
                            mybir.AluOpType.mult,
                        )
                        nc.vector.tensor_tensor(
                            stage_sb[64:128, qc % 2, hp, :],
                            hid[0:64, QC : 2 * QC],
                            rec[:, QC : 2 * QC],
                            mybir.AluOpType.mult,
                        )
                    else:
                        # one fast DVE copy releases the hid PSUM tile; the
                        # ln/exp/mult normalization runs via flush_norm()
                        raw = rawpool.tile([128, 2 * QC], BF16, tag="raw")
                        nc.vector.tensor_copy(raw[:], hid[:])
                        pending_norm.append((raw, qc, hp))
                    gidx += 1

            # epilogue: output projection of the last q-chunk
            flush_norm()
            for j in range(HP):
                tasks.append((gidx, task_o(NQC - 1, j)))
            drain_older(gidx + 1)
    return nc


def _make_tri():
    import ml_dtypes

    m = np.zeros((128, 256), dtype=np.float32)
    kk = np.arange(128)[:, None]
    qq = np.arange(128)[None, :]
    m[:, 128:256] = (kk <= qq).astype(np.float32)
    return np.ascontiguousarray(m).astype(ml_dtypes.bfloat16)


def _make_mask():
    import ml_dtypes

    m = np.zeros((128, 4, QC), dtype=np.float32)
    for rr in range(4):
        kk = np.arange(128)[:, None]
        qq = np.arange(QC)[None, :]
        m[:, rr, :] = (128 * rr + kk <= qq).astype(np.float32)
    return np.ascontiguousarray(m.reshape(128, 4 * QC)).astype(ml_dtypes.bfloat16)


def _pack8(a):
    """[E, X] -> [128, 4, 2, X] fp8e4m3 with E = e4*256 + j*128 + p, the
    DoubleRow contraction-pair layout (pairing must match between both
    matmul operands; it does by construction)."""
    import ml_dtypes

    r = np.ascontiguousarray(
        np.asarray(a, dtype=np.float32).reshape(4, 2, 128, -1).transpose(2, 0, 1, 3)
    )
    return r.astype(ml_dtypes.float8_e4m3fn)


def make_in_maps(x, W_q, W_k, W_v, W_o):
    import ml_dtypes

    bf16 = ml_dtypes.bfloat16
    tri = _make_tri()
    mask = _make_mask()
    in_maps = []
    for i in range(N_CORES):
        b, g = i // 2, i % 2
        xTb = np.ascontiguousarray(np.asarray(x)[b].T)
        m = {
            "xT": xTb.astype(bf16),
            "Wq": np.ascontiguousarray(
                np.asarray(W_q)[:, g * GF : (g + 1) * GF]
            ).astype(bf16),
            "Wk": np.ascontiguousarray(
                np.asarray(W_k)[:, g * GF : (g + 1) * GF]
            ).astype(bf16),
            "Wv": np.ascontiguousarray(
                np.asarray(W_v)[:, g * GF : (g + 1) * GF]
            ).astype(bf16),
            "Wo": np.ascontiguousarray(
                np.asarray(W_o)[g * GF : (g + 1) * GF, :]
            ).astype(bf16),
            "tri": tri,
            "mask": mask,
        }
        if USE_FP8_QK:
            m["xq8"] = _pack8(xTb)
            m["wq8"] = _pack8(np.asarray(W_q)[:, g * GF : (g + 1) * GF] * 32.0)
            m["wk8"] = _pack8(np.asarray(W_k)[:, g * GF : (g + 1) * GF] * 32.0)
        in_maps.append(m)
    return in_maps


def kernel(x, W_q, W_k, W_v, W_o):
    global _CACHED_NC
    from concourse.bass_utils import run_bass_kernel_spmd

    if _CACHED_NC is None:
        _CACHED_NC = build_nc()
    nc = _CACHED_NC

    in_maps = make_in_maps(x, W_q, W_k, W_v, W_o)
    res = run_bass_kernel_spmd(nc, in_maps, core_ids=list(range(N_CORES)))
    out = np.empty((B, C, E), dtype=np.float32)
    for b in range(B):
        out[b] = res.results[2 * b]["out"] + res.results[2 * b + 1]["out"]
    return out
